# revision 1
# baseline (speedup 1.0000x reference)
"""Trainium2 Bass kernel for nn_BotUpSaliency (B=2, H=W=512, K=12, 16 steps).

Math
----
The reference integrates, for 16 Euler steps (EPS=0.01):

    y'  = y + EPS*(-y + gx + conv(gx,W) + 1)
    x'  = x + EPS*(J0*gx + conv(gx,J) + inputs + i_norm - x - gy - gy@psi)
    gx  = clip(x - 1, 0, 1),  gy piecewise-linear,  out = mean_t gx_t, max over K

with x0 = 0.01, y0 = 1.  While gx == 0 (everywhere), the system collapses
exactly:
  * y stays exactly 1.0  (y + 0.01*(-1 + 0 + 0 + 1) == y), so gy == 0.21.
  * i_norm == 0.85 (conv of the all-zero s), conv(gx,*) == 0.
  * x_t = a_t * inputs + b_t elementwise, with scalar recurrences
        a_{t+1} = (1-EPS) a_t + EPS,           a_0 = 0
        b_{t+1} = (1-EPS) b_t + EPS*(0.85 - gy - colsum(psi)*gy),  b_0 = 0.01
  * gx_t = clip(a_t*inputs + b_t - 1, 0, 1) stays identically 0 as long as
        max_t (a_t * inputs.max() + b_t) < 1
    which requires inputs.max() >= ~6.66; the model's input domain is [0,1).

Hence out = (1/16) * sum_t clip(a_t*inputs + b_t - 1, 0, 1), and because each
term is nondecreasing in the input value, max over channels commutes with the
whole expression: it is evaluated at m = max_k inputs.

The device kernel computes exactly that: m = channel-max of the input slab
(reads all input bytes - the memory-bound part), then evaluates the sum of
affine-clip terms. Because the clip knots (1-b_t)/a_t decrease with t, for
m < (1-b_15)/a_15 ~= 7.075 the sum equals its t=16 term alone, and that term
stays below 1/16 there, so a single relu-affine evaluates it exactly on the
guard-certified domain.

A host-side guard verifies the collapse precondition (with wide margin) from
the actual inputs/psi and otherwise falls back to a full jax implementation
of the reference on CPU.

Sharding: pure data parallelism, 8 cores x 128 rows of the flattened
(2*512, 512, 12) input.
"""

import numpy as np

K = 12
STEPS = 16
EPS = 0.01
TX = 1.0
G1 = 0.21
J0 = 0.8
B, H, WD = 2, 512, 512
N_CORES = 8
ROWS = B * H                  # 1024 flattened rows
RPC = ROWS // N_CORES         # 128 rows per core == SBUF partitions
ROWW = WD * K                  # 6144 floats per row
# input is staged channel-major (host transpose): 12 planes of [rows, 512];
# DMA chunks of 2 planes pipeline across the two HWDGE rings
CHUNK_PLANES = (2, 2, 2, 2, 1, 1, 1, 1)
assert sum(CHUNK_PLANES) == K

_CACHE = {}


def _coeffs(colsum):
    """Scalar affine recurrence coefficients while gx == 0 (float64)."""
    gy = G1 * 1.0             # y stays exactly 1.0
    drive = 0.85 - gy - colsum * gy
    a, b = 0.0, 0.01
    A, Bc = [], []
    for _ in range(STEPS):
        a = (1.0 - EPS) * a + EPS
        b = (1.0 - EPS) * b + EPS * drive
        A.append(a)
        Bc.append(b)
    return np.array(A), np.array(Bc)


def _build_program(A, Bc):
    import concourse.bacc as bacc
    import concourse.mybir as mybir
    from concourse.tile import TileContext

    f32 = mybir.dt.float32
    bf16 = mybir.dt.bfloat16
    relu = mybir.ActivationFunctionType.Relu

    nc = bacc.Bacc("TRN2", target_bir_lowering=False, debug=False)
    x = nc.dram_tensor("x", [RPC, ROWW], bf16, kind="ExternalInput")
    out = nc.dram_tensor("out", [RPC, WD], f32, kind="ExternalOutput")

    with TileContext(nc) as tc:
        with (
            tc.tile_pool(name="inp", bufs=8) as inpool,
            tc.tile_pool(name="zs", bufs=8) as zpool,
            tc.tile_pool(name="one", bufs=1) as spool,
        ):
            # per-step activation biases (b_t - 1)/16 as [128,1] scalars; the
            # 1/16 out-scale is folded into scale/bias/min so acc IS the output
            btab = spool.tile([RPC, STEPS], f32, tag="btab")
            for s in range(STEPS):
                nc.gpsimd.memset(btab[:, s:s + 1], float((Bc[s] - 1.0) / STEPS))
            # warm the ACT Relu table during the DMA window (1.3us table load)
            zw = spool.tile([RPC, 1], f32, tag="zw")
            nc.scalar.activation(out=zw[:], in_=btab[:, 0:1], func=relu)
            # m = per-pixel channel max: pairwise bf16 TT maxes over channel
            # planes (334ns each in 2x mode vs ~1.5us strided reduce); chunk
            # DMAs alternate between the two HWDGE rings (SP + ACT)
            m = spool.tile([RPC, WD], bf16, tag="m")
            running = None
            col = 0
            for c, npl in enumerate(CHUNK_PLANES):
                t = inpool.tile([RPC, npl * WD], bf16, tag=f"in{npl}", name="t")
                dma_eng = nc.sync if c % 2 == 0 else nc.scalar
                dma_eng.dma_start(out=t[:], in_=x[:, col:col + npl * WD])
                col += npl * WD
                last = c == len(CHUNK_PLANES) - 1
                if npl == 2:
                    p = zpool.tile([RPC, WD], bf16, tag="pp", name="pp")
                    nc.vector.tensor_tensor(
                        out=p[:], in0=t[:, :WD], in1=t[:, WD:],
                        op=mybir.AluOpType.max)
                else:
                    p = t
                if running is None:
                    running = p
                else:
                    nxt = m if last else zpool.tile(
                        [RPC, WD], bf16, tag="rm", name="rm")
                    nc.vector.tensor_tensor(
                        out=nxt[:], in0=running[:], in1=p[:, :WD],
                        op=mybir.AluOpType.max)
                    running = nxt
            # acc = sum_t clip(a_t*m + b_t - 1, 0, 1); ACT does the affine+relu,
            # DVE runs the fused (min 1) + acc chain (STT is DVE-only; Pool's
            # tensor_scalar ucode is ~15x slower and contends on the SBUF port).
            # The clip knots (1-b_t)/a_t DECREASE with t, so for
            # m < (1-b_15)/a_15 ~= 7.075 only the t=16 term can be nonzero and
            # sum_t clip(a_t*m + b_t - 1, 0, 1) == clip(a_16*m + b_16 - 1, 0, 1)
            # exactly. On that whole domain the term tops out at ~0.061 < 1,
            # so the upper clip can never bind either: one ACT relu-affine IS
            # the exact result (1/16 folded in). Host guard certifies m < 6.53.
            z = spool.tile([RPC, WD], f32, tag="z")
            nc.scalar.activation(
                out=z[:], in_=m[:], func=relu,
                bias=btab[:, STEPS - 1:STEPS], scale=float(A[STEPS - 1] / STEPS),
            )
            half = WD // 2
            nc.sync.dma_start(out=out[:, :half], in_=z[:, :half])
            nc.scalar.dma_start(out=out[:, half:], in_=z[:, half:])

    nc.compile()
    return nc


def _get_program(A, Bc):
    key = (tuple(np.round(A, 12)), tuple(np.round(Bc, 12)))
    if key not in _CACHE:
        _CACHE[key] = _build_program(A, Bc)
    return _CACHE[key]


def _run_on_device(inputs_np, A, Bc, trace=False):
    from concourse.bass_utils import run_bass_kernel_spmd

    nc = _get_program(A, Bc)
    import ml_dtypes
    flat = np.ascontiguousarray(
        inputs_np.reshape(ROWS, WD, K).transpose(0, 2, 1)
    ).astype(ml_dtypes.bfloat16).reshape(ROWS, ROWW)
    in_maps = [
        {"x": np.ascontiguousarray(flat[i * RPC:(i + 1) * RPC])}
        for i in range(N_CORES)
    ]
    res = run_bass_kernel_spmd(nc, in_maps, list(range(N_CORES)), trace=trace)
    out = np.concatenate([res.results[i]["out"] for i in range(N_CORES)], axis=0)
    return out.reshape(B, H, WD).astype(np.float32), res


def _reference_fallback(inputs, Wk, Jk, psi):
    """Full reference math in jax on CPU (only for out-of-domain inputs)."""
    import jax
    import jax.numpy as jnp

    cpu = jax.devices("cpu")[0]
    with jax.default_device(cpu):
        inputs = jnp.asarray(np.asarray(inputs), jnp.float32)
        Wk = jnp.asarray(np.asarray(Wk), jnp.float32)
        Jk = jnp.asarray(np.asarray(Jk), jnp.float32)
        psi = jnp.asarray(np.asarray(psi), jnp.float32)
        PAD = 7

        def _conv(xx, kk, padding):
            return jax.lax.conv_general_dilated(
                xx, kk, (1, 1), padding,
                dimension_numbers=("NHWC", "HWIO", "NHWC"))

        def _gx(xx):
            return jnp.clip(xx - TX, 0.0, 1.0)

        def _gy(yy):
            yc = jnp.maximum(yy, 0.0)
            return jnp.where(yc <= 1.2, G1 * yc, G1 * 1.2 + 2.5 * (yc - 1.2))

        psi_mat = psi[0, 0]
        box = jnp.ones((5, 5, 1, 1), inputs.dtype)
        x = jnp.full_like(inputs, 0.01)
        y = jnp.ones_like(inputs)
        gx = _gx(x)
        gy = _gy(y)
        out = jnp.zeros_like(inputs)
        for _ in range(STEPS):
            s = jnp.sum(gx, axis=3, keepdims=True)
            i_norm = 0.85 - 2.0 * (_conv(s, box, "SAME") / 25.0) ** 2
            gx_p = jnp.pad(gx, ((0, 0), (PAD, PAD), (PAD, PAD), (0, 0)),
                           mode="symmetric")
            inhib = _conv(gx_p, Wk, "VALID")
            excit = _conv(gx_p, Jk, "VALID")
            inhibs_psi = jnp.einsum("bhwi,io->bhwo", gy, psi_mat)
            y_new = y + EPS * (-y + gx + inhib + 1.0)
            x_inhib = x + gy + inhibs_psi
            x_excit = J0 * gx + excit + inputs + i_norm
            x_new = x + EPS * (x_excit - x_inhib)
            gx = _gx(x_new)
            gy = _gy(y_new)
            x, y = x_new, y_new
            out = out + gx
        out = out / STEPS
        return np.asarray(jnp.max(out, axis=3))


def kernel(inputs, W=None, J=None, psi=None, **_ignored):
    inputs_np = np.asarray(inputs, dtype=np.float32)
    assert inputs_np.shape == (B, H, WD, K), inputs_np.shape

    # Guard: the gx==0 collapse must hold for these inputs/psi.
    ok = True
    colsum = 3.0
    if psi is not None:
        cs = np.asarray(psi, dtype=np.float64)[0, 0].sum(axis=0)
        if np.max(np.abs(cs - cs[0])) < 1e-9:
            colsum = float(cs[0])
        else:
            ok = False
    if ok:
        A, Bc = _coeffs(colsum)
        # 1.004 factor covers bf16 round-up of the staged inputs (<= 2^-8 rel)
        mx = float(inputs_np.max()) * 1.004
        if np.max(A * mx + Bc) >= 0.98:
            ok = False
    if not ok:
        return _reference_fallback(inputs, W, J, psi).astype(np.float32)

    out, _ = _run_on_device(inputs_np, A, Bc)
    return out


if __name__ == "__main__":
    rng = np.random.default_rng(0)
    x = rng.random((B, H, WD, K), dtype=np.float32)
    o = kernel(inputs=x)
    print("kernel out:", o.shape, o.dtype, "maxabs", np.abs(o).max())



# revision 2
# speedup vs baseline: 1.0333x; 1.0333x over previous
"""Trainium2 Bass kernel for nn_BotUpSaliency (B=2, H=W=512, K=12, 16 steps).

Math
----
The reference integrates, for 16 Euler steps (EPS=0.01):

    y'  = y + EPS*(-y + gx + conv(gx,W) + 1)
    x'  = x + EPS*(J0*gx + conv(gx,J) + inputs + i_norm - x - gy - gy@psi)
    gx  = clip(x - 1, 0, 1),  gy piecewise-linear,  out = mean_t gx_t, max over K

with x0 = 0.01, y0 = 1.  While gx == 0 (everywhere), the system collapses
exactly:
  * y stays exactly 1.0  (y + 0.01*(-1 + 0 + 0 + 1) == y), so gy == 0.21.
  * i_norm == 0.85 (conv of the all-zero s), conv(gx,*) == 0.
  * x_t = a_t * inputs + b_t elementwise, with scalar recurrences
        a_{t+1} = (1-EPS) a_t + EPS,           a_0 = 0
        b_{t+1} = (1-EPS) b_t + EPS*(0.85 - gy - colsum(psi)*gy),  b_0 = 0.01
  * gx_t = clip(a_t*inputs + b_t - 1, 0, 1) stays identically 0 as long as
        max_t (a_t * inputs.max() + b_t) < 1
    which requires inputs.max() >= ~6.66; the model's input domain is [0,1).

Hence out = (1/16) * sum_t clip(a_t*inputs + b_t - 1, 0, 1), and because each
term is nondecreasing in the input value, max over channels commutes with the
whole expression: it is evaluated at m = max_k inputs.  Because the clip
knots (1-b_t)/a_t decrease with t, for m < (1-b_15)/a_15 ~= 6.66 the sum
equals its t=16 term alone and that term stays below 1/16 and above -inf, so
a single affine + relu evaluates it exactly on the guard-certified domain.

The device kernel computes exactly that: m = per-pixel channel max of the
input slab (reads all input bytes - the memory-bound part), then
z = relu(scale*m + bias) with the 1/16 out-scale folded in.

A host-side guard verifies the collapse precondition (with wide margin) from
the actual inputs/psi and otherwise falls back to a full jax implementation
of the reference on CPU.

Device schedule (raw Bass, no Tile framework)
---------------------------------------------
Measured floor: the NEFF pays a fixed ~8us epilogue (walrus's per-engine
semaphore-file sweep) plus ~1us preamble, and 1.5MB/core of bf16 input at
the 8-core-shared HBM rate (~250-350 GB/s effective per core).  The schedule
minimizes everything else:
  * raw Bass (no TileContext) starts the first input DMA ~0.3us earlier and
    makes same-queue waits cost ~35ns instead of ~200ns.
  * input is plane-major bf16; ring A (sync HWDGE) carries planes 0-5,
    ring B (scalar HWDGE) planes 6-11, three 256KB chunks each.
  * DVE folds pairs of 2-plane chunks at 2x mode ([128,1024] tensor_tensor
    max, ~690ns) as chunks land; 6 fold ops total.
  * no Activation engine at all: the final z = relu(scale*m + bias) runs as
    two DVE tensor_scalar ops (4x mode, ~293ns each).  This removes the
    1.28us ACT table load that otherwise delays ring B's descriptor issue.
  * output is written as bf16 (exact: all zeros on the certified domain)
    and upcast on host; halves split across both rings.

Sharding: pure data parallelism, 8 cores x 128 rows of the flattened
(2*512, 512, 12) input.
"""

import numpy as np

K = 12
STEPS = 16
EPS = 0.01
TX = 1.0
G1 = 0.21
J0 = 0.8
B, H, WD = 2, 512, 512
N_CORES = 8
ROWS = B * H                  # 1024 flattened rows
RPC = ROWS // N_CORES         # 128 rows per core == SBUF partitions
ROWW = WD * K                 # 6144 bf16 per row, plane-major

_CACHE = {}


def _coeffs(colsum):
    """Scalar affine recurrence coefficients while gx == 0 (float64)."""
    gy = G1 * 1.0             # y stays exactly 1.0
    drive = 0.85 - gy - colsum * gy
    a, b = 0.0, 0.01
    A, Bc = [], []
    for _ in range(STEPS):
        a = (1.0 - EPS) * a + EPS
        b = (1.0 - EPS) * b + EPS * drive
        A.append(a)
        Bc.append(b)
    return np.array(A), np.array(Bc)


def _build_program(A, Bc):
    import concourse.bacc as bacc
    import concourse.mybir as mybir

    bf16 = mybir.dt.bfloat16
    mx = mybir.AluOpType.max
    P = WD
    scale = float(A[STEPS - 1] / STEPS)
    bias = float((Bc[STEPS - 1] - 1.0) / STEPS)

    nc = bacc.Bacc("TRN2", target_bir_lowering=False, debug=False)
    x = nc.dram_tensor("x", [RPC, ROWW], bf16, kind="ExternalInput")
    out = nc.dram_tensor("out", [RPC, WD], bf16, kind="ExternalOutput")

    t = nc.alloc_sbuf_tensor("t", [RPC, ROWW], bf16)
    o1 = nc.alloc_sbuf_tensor("o1", [RPC, 2 * P], bf16)
    o2 = nc.alloc_sbuf_tensor("o2", [RPC, 2 * P], bf16)
    o3 = nc.alloc_sbuf_tensor("o3", [RPC, 2 * P], bf16)
    o4 = nc.alloc_sbuf_tensor("o4", [RPC, 2 * P], bf16)
    o5 = nc.alloc_sbuf_tensor("o5", [RPC, 2 * P], bf16)
    m = nc.alloc_sbuf_tensor("m", [RPC, P], bf16)
    z1 = nc.alloc_sbuf_tensor("z1", [RPC, P], bf16)
    z = nc.alloc_sbuf_tensor("z", [RPC, P], bf16)

    # one completion semaphore per chunk: a single shared counter would race
    # (each DMA is split over 16 SDMA engines; +16 can be reached by a mix
    # of two chunks' slices)
    sas = [nc.alloc_semaphore(f"sa{i}") for i in range(3)]
    sbs = [nc.alloc_semaphore(f"sb{i}") for i in range(3)]
    sv = nc.alloc_semaphore("sv")   # DVE progress (engine write->read ordering)
    sd = nc.alloc_semaphore("sd")   # z ready for out-DMA
    oa = nc.alloc_semaphore("oa")
    ob = nc.alloc_semaphore("ob")

    tp = t.ap()
    seg = lambda i, n: tp[:, i * P:(i + n) * P]

    for ci, s in enumerate((0, 2, 4)):
        nc.sync.dma_start(out=seg(s, 2), in_=x[:, s * P:(s + 2) * P]).then_inc(sas[ci], 16)
    for ci, s in enumerate((6, 8, 10)):
        nc.scalar.dma_start(out=seg(s, 2), in_=x[:, s * P:(s + 2) * P]).then_inc(sbs[ci], 16)

    v = nc.vector
    v.wait_ge(sas[0], 16)
    v.wait_ge(sbs[0], 16)
    v.tensor_tensor(out=o1.ap(), in0=seg(0, 2), in1=seg(6, 2), op=mx).then_inc(sv, 1)
    v.wait_ge(sas[1], 16)
    v.wait_ge(sbs[1], 16)
    v.tensor_tensor(out=o2.ap(), in0=seg(2, 2), in1=seg(8, 2), op=mx).then_inc(sv, 1)
    v.wait_ge(sv, 2)
    v.tensor_tensor(out=o3.ap(), in0=o1.ap(), in1=o2.ap(), op=mx).then_inc(sv, 1)
    v.wait_ge(sas[2], 16)
    v.wait_ge(sbs[2], 16)
    v.tensor_tensor(out=o4.ap(), in0=seg(4, 2), in1=seg(10, 2), op=mx).then_inc(sv, 1)
    v.wait_ge(sv, 4)
    v.tensor_tensor(out=o5.ap(), in0=o3.ap(), in1=o4.ap(), op=mx).then_inc(sv, 1)
    v.wait_ge(sv, 5)
    v.tensor_tensor(out=m.ap(), in0=o5.ap()[:, :P], in1=o5.ap()[:, P:], op=mx).then_inc(sv, 1)
    # z = relu(scale*m + bias); two-stage form keeps the result exactly +-0.0
    # on the certified domain (scale*m + bias < 0 there)
    v.wait_ge(sv, 6)
    v.tensor_scalar(out=z1.ap(), in0=m.ap(), scalar1=scale, scalar2=bias,
                    op0=mybir.AluOpType.mult, op1=mybir.AluOpType.add).then_inc(sv, 1)
    v.wait_ge(sv, 7)
    v.tensor_scalar(out=z.ap(), in0=z1.ap(), scalar1=0.0, scalar2=None,
                    op0=mx).then_inc(sd, 1)

    half = WD // 2
    nc.sync.wait_ge(sd, 1)
    nc.sync.dma_start(out=out[:, :half], in_=z.ap()[:, :half]).then_inc(oa, 16)
    nc.scalar.wait_ge(sd, 1)
    nc.scalar.dma_start(out=out[:, half:], in_=z.ap()[:, half:]).then_inc(ob, 16)
    nc.sync.wait_ge(oa, 16)
    nc.scalar.wait_ge(ob, 16)

    nc.compile()
    return nc


def _get_program(A, Bc):
    key = (tuple(np.round(A, 12)), tuple(np.round(Bc, 12)))
    if key not in _CACHE:
        _CACHE[key] = _build_program(A, Bc)
    return _CACHE[key]


def _run_on_device(inputs_np, A, Bc, trace=False):
    from concourse.bass_utils import run_bass_kernel_spmd

    nc = _get_program(A, Bc)
    import ml_dtypes
    flat = np.ascontiguousarray(
        inputs_np.reshape(ROWS, WD, K).transpose(0, 2, 1)
    ).astype(ml_dtypes.bfloat16).reshape(ROWS, ROWW)
    in_maps = [
        {"x": np.ascontiguousarray(flat[i * RPC:(i + 1) * RPC])}
        for i in range(N_CORES)
    ]
    res = run_bass_kernel_spmd(nc, in_maps, list(range(N_CORES)), trace=trace)
    out = np.concatenate(
        [np.asarray(res.results[i]["out"]) for i in range(N_CORES)], axis=0)
    return out.reshape(B, H, WD).astype(np.float32), res


def _reference_fallback(inputs, Wk, Jk, psi):
    """Full reference math in jax on CPU (only for out-of-domain inputs)."""
    import jax
    import jax.numpy as jnp

    cpu = jax.devices("cpu")[0]
    with jax.default_device(cpu):
        inputs = jnp.asarray(np.asarray(inputs), jnp.float32)
        Wk = jnp.asarray(np.asarray(Wk), jnp.float32)
        Jk = jnp.asarray(np.asarray(Jk), jnp.float32)
        psi = jnp.asarray(np.asarray(psi), jnp.float32)
        PAD = 7

        def _conv(xx, kk, padding):
            return jax.lax.conv_general_dilated(
                xx, kk, (1, 1), padding,
                dimension_numbers=("NHWC", "HWIO", "NHWC"))

        def _gx(xx):
            return jnp.clip(xx - TX, 0.0, 1.0)

        def _gy(yy):
            yc = jnp.maximum(yy, 0.0)
            return jnp.where(yc <= 1.2, G1 * yc, G1 * 1.2 + 2.5 * (yc - 1.2))

        psi_mat = psi[0, 0]
        box = jnp.ones((5, 5, 1, 1), inputs.dtype)
        x = jnp.full_like(inputs, 0.01)
        y = jnp.ones_like(inputs)
        gx = _gx(x)
        gy = _gy(y)
        out = jnp.zeros_like(inputs)
        for _ in range(STEPS):
            s = jnp.sum(gx, axis=3, keepdims=True)
            i_norm = 0.85 - 2.0 * (_conv(s, box, "SAME") / 25.0) ** 2
            gx_p = jnp.pad(gx, ((0, 0), (PAD, PAD), (PAD, PAD), (0, 0)),
                           mode="symmetric")
            inhib = _conv(gx_p, Wk, "VALID")
            excit = _conv(gx_p, Jk, "VALID")
            inhibs_psi = jnp.einsum("bhwi,io->bhwo", gy, psi_mat)
            y_new = y + EPS * (-y + gx + inhib + 1.0)
            x_inhib = x + gy + inhibs_psi
            x_excit = J0 * gx + excit + inputs + i_norm
            x_new = x + EPS * (x_excit - x_inhib)
            gx = _gx(x_new)
            gy = _gy(y_new)
            x, y = x_new, y_new
            out = out + gx
        out = out / STEPS
        return np.asarray(jnp.max(out, axis=3))


def kernel(inputs, W=None, J=None, psi=None, **_ignored):
    inputs_np = np.asarray(inputs, dtype=np.float32)
    assert inputs_np.shape == (B, H, WD, K), inputs_np.shape

    # Guard: the gx==0 collapse must hold for these inputs/psi.
    ok = True
    colsum = 3.0
    if psi is not None:
        cs = np.asarray(psi, dtype=np.float64)[0, 0].sum(axis=0)
        if np.max(np.abs(cs - cs[0])) < 1e-9:
            colsum = float(cs[0])
        else:
            ok = False
    if ok:
        A, Bc = _coeffs(colsum)
        # 1.004 factor covers bf16 round-up of the staged inputs (<= 2^-8 rel)
        mx = float(inputs_np.max()) * 1.004
        if np.max(A * mx + Bc) >= 0.98:
            ok = False
    if not ok:
        return _reference_fallback(inputs, W, J, psi).astype(np.float32)

    out, _ = _run_on_device(inputs_np, A, Bc)
    return out


if __name__ == "__main__":
    rng = np.random.default_rng(0)
    x = rng.random((B, H, WD, K), dtype=np.float32)
    o = kernel(inputs=x)
    print("kernel out:", o.shape, o.dtype, "maxabs", np.abs(o).max())
